# revision 34
# baseline (speedup 1.0000x reference)
"""Trainium2 Bass kernel for nn_FCond (FiLM-conditioned MLP chain).

Reference computation (B=32, N=100000, D=3, CDIM=128):
    h = x
    for kblk in [0, 1, 2, 2, 2, 2]:
        h = tanh((h @ Wk.T + bk) * sigmoid(c @ Wsk.T + bsk) + (c @ Wbk.T + bbk))

Since the FiLM conditioning depends only on (c, weights), each (batch,
block) reduces to an affine map  h' = tanh(A_kb @ h + d_kb)  with
A_kb [3,3], d_kb [3] precomputed on the host in float64.

Device strategy (pure data parallel over 8 cores, 4 batches/core):
  - Layout: partition p = b*32 + comp*10 + g  (4 batch-bands of 32
    partitions; 3 comps x 10 point-groups per band; rows 30,31 of each
    band are zero padding). Free dim = 10240 points per (b,comp,g)
    stream (N padded 100000 -> 102400).
  - Each block is ONE block-diagonal [128x128] matmul on TensorE
    (40 real points per column), PSUM accumulated, then ScalarE does
    tanh(psum + d) with a per-partition bias AP, evacuating PSUM->SBUF.
  - 5 chunks of 2048 columns stream through DMA-in -> 6 blocks -> DMA-out,
    interleaved in groups of 3 chunks so consecutive PE matmul groups come
    from independent chunks (ScalarE tanh is the bottleneck engine; the
    PE/ACT chain stays dense). A 16-matmul warmup burst flips the PE HAM
    clock gate to 2.4 GHz before the main chain.

MM_DTYPE: float32r (TF32-like reduced-precision PE mode, ~1.9 cyc/col
warm) vs float32 (exact, ~4.5 cyc/col). Selected by MM_EXACT below.
Measured end-to-end rel err vs the fp32 reference: 8.8e-5 (f32r).
"""
import sys
import types

import numpy as np

B, N, D, CDIM = 32, 100000, 3, 128
NCORES = 8
BPC = B // NCORES          # batches per core
G = 10                     # point-groups per (batch, comp)
L = 10000                  # points per partition stream (N / G, exact)
NPAD = G * L               # = N exactly (no padding)
P = 128                    # partitions
CHUNK = 2048               # free-dim chunk (4 PSUM banks fp32)
MM_F = 512                 # matmul free chunk (1 PSUM bank)
NCHUNK = 5                 # chunk sizes: 4x2048 + 1808

MM_EXACT = False           # True -> float32 matmuls (exact, ~2.9x slower PE)
RAW = False                # hand-scheduled pipeline (no TileContext); Tile version is faster

PROFILE = False            # set by test harness; collects HW exec time
LAST_EXEC_NS = None

_CACHE = {}


def _install_profile_shim():
    """Register the NTFF profile hook (missing antenv.axon_hooks in this
    container) so run_bass_kernel_spmd(trace=True) can report exec time."""
    if "antenv.axon_hooks" in sys.modules:
        return
    mod = types.ModuleType("antenv.axon_hooks")
    _state = {"hook": None}
    mod.set_axon_ntff_profile_hook = lambda h: _state.__setitem__("hook", h)
    mod.get_axon_ntff_profile_hook = lambda: _state["hook"]
    sys.modules["antenv.axon_hooks"] = mod
    try:
        from trn_agent_boot.trn_boot import _ntff_profile_via_ctypes
        mod.set_axon_ntff_profile_hook(
            _ntff_profile_via_ctypes("/opt/axon/libaxon_pjrt.so"))
    except Exception:
        pass
    import concourse.bass_utils as bu
    bu.upload_artifacts = lambda tmpdir: f"local:{tmpdir}"


def _build_program():
    import concourse.bacc as bacc
    import concourse.tile as tile
    from concourse import mybir

    f32 = mybir.dt.float32
    mmdt = f32 if MM_EXACT else mybir.dt.float32r
    Tanh = mybir.ActivationFunctionType.Tanh
    Copy = mybir.ActivationFunctionType.Copy
    WSETS = (0, 1, 2, 2, 2, 2)

    nc = bacc.Bacc("TRN2", target_bir_lowering=False, debug=False)
    x_d = nc.declare_dram_parameter("x", [P, L], f32, isOutput=False)
    w_d = nc.declare_dram_parameter("w", [3, P, P], f32, isOutput=False)
    d_d = nc.declare_dram_parameter("d", [P, 3], f32, isOutput=False)
    y_d = nc.declare_dram_parameter("y", [P, L], f32, isOutput=True)

    with tile.TileContext(nc) as tc:
        with (
            tc.tile_pool(name="wpool", bufs=1) as wpool,
            tc.tile_pool(name="xinpool", bufs=5) as xinpool,
            tc.tile_pool(name="youtpool", bufs=5) as youtpool,
            tc.tile_pool(name="hpool", bufs=6) as hpool,
            tc.tile_pool(name="psum", bufs=2, space="PSUM") as psum,
        ):
            # --- first compute chunk's DMA goes out before anything else
            # so the PE/ACT chain can start ASAP. ---
            h0 = xinpool.tile([P, CHUNK], mmdt, name="xin0", tag="xin")
            nc.sync.dma_start(h0[:], x_d[:, 0:CHUNK].bitcast(mmdt))

            # --- weights/bias: DMA once, one ACT-copy (f32r rounding +
            # makes matmul weight input ACT-produced). ---
            bias = wpool.tile([P, 3], f32)
            nc.sync.dma_start(bias[:], d_d[:])
            wraw = wpool.tile([P, 3 * P], f32, name="wraw", tag="wraw")
            for k in range(3):
                nc.sync.dma_start(wraw[:, k * P:(k + 1) * P], w_d[k])
            wall = wpool.tile([P, 3 * P], mmdt, name="wall", tag="wall")
            nc.scalar.activation(wall[:], wraw[:], Copy)
            wts = [wall[:, k * P:(k + 1) * P] for k in range(3)]

            # PE warmup burst: ~16 dense matmuls (~4us) to flip the HAM
            # clock gate to 2.4 GHz before the main chain; runs while the
            # input DMAs stream in.
            warm0 = wpool.tile([P, MM_F], f32, name="warm0", tag="warm0")
            nc.vector.memset(warm0[:], 0.0)
            # dummy tanh: pulls the ACT table load off the critical chain
            nc.scalar.activation(warm0[:, 0:1], warm0[:, 0:1], Tanh,
                                 bias=0.0, scale=1.0)
            warm_src = wpool.tile([P, MM_F], mmdt, name="warmsrc",
                                  tag="warmsrc")
            nc.scalar.activation(warm_src[:], warm0[:], Copy)
            warm_ps = psum.tile([P, MM_F], f32, name="warmps", tag="ps")
            for _ in range(16):
                nc.tensor.matmul(warm_ps[:], warm_src[:, 0:P], warm_src[:],
                                 start=True, stop=True)

            # Chunk-group software pipeline: within a group, consecutive
            # matmul groups come from rotating chunks, so each group's
            # dependency on the previous block's tanh has >=2 matmul
            # groups of slack and the PE streams. First chunk is small so
            # the chain starts as soon as its DMA lands.
            sizes = [CHUNK] * (NCHUNK - 1) + [L - CHUNK * (NCHUNK - 1)]
            offs = [sum(sizes[:i]) for i in range(len(sizes))]
            groups = [(0, 1), (2, 3, 4)]
            hs = {}
            hs[0] = h0
            for grp in groups:
                for ci in grp:
                    if ci == 0:
                        continue
                    h = xinpool.tile([P, sizes[ci]], mmdt, name=f"xin{ci}",
                                     tag="xin")
                    nc.sync.dma_start(
                        h[:],
                        x_d[:, offs[ci]:offs[ci] + sizes[ci]].bitcast(mmdt))
                    hs[ci] = h
                for kblk in range(6):
                    ks = WSETS[kblk]
                    last = kblk == 5
                    for ci in grp:
                        sz = sizes[ci]
                        ps = psum.tile([P, sz], f32,
                                       name=f"ps{ci}_{kblk}", tag="ps")
                        for j in range(0, sz, MM_F):
                            je = min(j + MM_F, sz)
                            nc.tensor.matmul(
                                ps[:, j:je],
                                wts[ks],
                                hs[ci][:, j:je],
                                start=True, stop=True)
                        hn = (youtpool.tile([P, sz], f32,
                                            name=f"yo{ci}", tag="yout")
                              if last else
                              hpool.tile([P, sz], mmdt,
                                         name=f"h{ci}_{kblk}", tag="h"))
                        nc.scalar.activation(hn[:], ps[:], Tanh,
                                             bias=bias[:, ks:ks + 1],
                                             scale=1.0)
                        hs[ci] = hn
                        if last:
                            c0 = offs[ci]
                            nc.sync.dma_start(y_d[:, c0:c0 + sz], hn[:])
    nc.compile()
    return nc


def _build_program_raw():
    """Hand-scheduled variant (no TileContext): manual semaphores, static
    SBUF layout. Avoids Tile's ~2.7us preamble and ~13us epilogue
    (drain + all-engine barriers + 57-semaphore clear butterfly).

    Pipeline: 30 stages s = (group, kblk, chunk), chunk-interleaved as
    [(0,1,2),(3,4)]. PSUM ping-pongs 2 x [128,2048] (banks 0-3 / 4-7).
      PE  stage s: wait tanh(input stage) and tanh(s-2) (psum WAR), 4 MMs,
                   inc pe_sem.
      ACT stage s: wait pe_sem >= s+1, tanh(ps[s%2] + bias) -> h, inc.
      SYNC: all input DMAs up front; per-chunk output DMA after its last
            tanh; final wait for DMA completion.
    """
    import concourse.bass as bass
    import concourse.bacc as bacc
    from concourse import mybir

    f32 = mybir.dt.float32
    mmdt = f32 if MM_EXACT else mybir.dt.float32r
    Tanh = mybir.ActivationFunctionType.Tanh
    Copy = mybir.ActivationFunctionType.Copy
    WSETS = (0, 1, 2, 2, 2, 2)

    nc = bacc.Bacc("TRN2", target_bir_lowering=False, debug=False)
    x_d = nc.declare_dram_parameter("x", [P, L], f32, isOutput=False)
    w_d = nc.declare_dram_parameter("w", [3, P, P], f32, isOutput=False)
    d_d = nc.declare_dram_parameter("d", [P, 3], f32, isOutput=False)
    y_d = nc.declare_dram_parameter("y", [P, L], f32, isOutput=True)

    # static SBUF tensors
    xin = [nc.alloc_sbuf_tensor(f"xin{c}", [P, CHUNK], mmdt).ap()
           for c in range(NCHUNK)]
    ha = [nc.alloc_sbuf_tensor(f"ha{c}", [P, CHUNK], mmdt).ap()
          for c in range(NCHUNK)]
    hb = [nc.alloc_sbuf_tensor(f"hb{c}", [P, CHUNK], mmdt).ap()
          for c in range(NCHUNK)]
    yout = [nc.alloc_sbuf_tensor(f"yout{c}", [P, CHUNK], f32).ap()
            for c in range(NCHUNK)]
    wraw = nc.alloc_sbuf_tensor("wraw", [P, 3 * P], f32).ap()
    wall = nc.alloc_sbuf_tensor("wall", [P, 3 * P], mmdt).ap()
    biast = nc.alloc_sbuf_tensor("biast", [P, 3], f32).ap()
    ps = [nc.alloc_psum_tensor(f"ps{i}", [P, CHUNK], f32).ap()
          for i in range(2)]

    # stage enumeration
    stages = []
    for grp in ((0, 1, 2), (3, 4)):
        for kblk in range(6):
            for ci in grp:
                stages.append((kblk, ci))
    idx = {kc: s for s, kc in enumerate(stages)}

    # h buffer chain per chunk: block k's input; k=0 -> xin, then a/b
    def h_in(kblk, ci):
        if kblk == 0:
            return xin[ci]
        return ha[ci] if kblk % 2 == 1 else hb[ci]

    def h_out(kblk, ci):
        if kblk == 5:
            return yout[ci]
        return ha[ci] if kblk % 2 == 0 else hb[ci]

    with (
        nc.Block() as block,
        nc.semaphore("dma_x0") as dx0,
        nc.semaphore("dma_x1") as dx1,
        nc.semaphore("dma_x2") as dx2,
        nc.semaphore("dma_x3") as dx3,
        nc.semaphore("dma_x4") as dx4,
        nc.semaphore("dma_wd") as dwd,
        nc.semaphore("dma_out") as dout,
        nc.semaphore("act_sem") as act_sem,
        nc.semaphore("pe_sem") as pe_sem,
    ):
        dxs = [dx0, dx1, dx2, dx3, dx4]

        @block.sync
        def _(sync: bass.BassEngine):
            sync.dma_start(out=biast, in_=d_d[:]).then_inc(dwd, 16)
            for k in range(3):
                sync.dma_start(out=wraw[:, k * P:(k + 1) * P],
                               in_=w_d[k]).then_inc(dwd, 16)
            sync.dma_start(out=xin[0], in_=x_d[:, 0:CHUNK].bitcast(mmdt)
                           ).then_inc(dxs[0], 16)
            for c in range(1, NCHUNK):
                if c >= 2:
                    # bound in-flight HWDGE transfers (ring capacity)
                    sync.wait_ge(dxs[c - 2], 16)
                sync.dma_start(out=xin[c],
                               in_=x_d[:, c * CHUNK:(c + 1) * CHUNK]
                               .bitcast(mmdt)).then_inc(dxs[c], 16)
            for ci in range(NCHUNK):
                sync.dma_start(out=y_d[:, ci * CHUNK:(ci + 1) * CHUNK],
                               in_=h_out(5, ci)
                               )._wait_ge(act_sem, idx[(5, ci)] + 2
                                          ).then_inc(dout, 16)
            sync.wait_ge(dout, 16 * NCHUNK)

        @block.scalar
        def _(scalar: bass.BassEngine):
            scalar.wait_ge(dwd, 64)
            scalar.activation(wall, wraw, Copy).then_inc(act_sem, 1)
            for s, (kblk, ci) in enumerate(stages):
                ks = WSETS[kblk]
                out_ap = h_out(kblk, ci)
                scalar.activation(
                    out_ap, ps[s % 2], Tanh,
                    bias=biast[:, ks:ks + 1], scale=1.0,
                )._wait_ge(pe_sem, s + 1).then_inc(act_sem, 1)

        @block.tensor
        def _(tensor: bass.BassEngine):
            tensor.wait_ge(act_sem, 1)
            for _i in range(16):
                tensor.matmul(ps[0][:, 0:P], wall[:, 0:P], wall[:, 0:P],
                              start=True, stop=True)
            for s, (kblk, ci) in enumerate(stages):
                ks = WSETS[kblk]
                # input-ready + psum-WAR deps, folded to one act_sem wait
                a_req = 0
                if kblk > 0:
                    a_req = idx[(kblk - 1, ci)] + 2
                if s >= 2:
                    a_req = max(a_req, s)
                if kblk == 0:
                    tensor.wait_ge(dxs[ci], 16)
                rhs = h_in(kblk, ci)
                for j in range(0, CHUNK, MM_F):
                    mm = tensor.matmul(ps[s % 2][:, j:j + MM_F],
                                       wall[:, ks * P:(ks + 1) * P],
                                       rhs[:, j:j + MM_F],
                                       start=True, stop=True)
                    if j == 0 and a_req:
                        mm._wait_ge(act_sem, a_req)
                mm.then_inc(pe_sem, 1)

    nc.compile()
    return nc


def _film_params(c, Wk, bk, Wsk, bsk, Wbk, bbk):
    """A[b] = diag(scale[b]) @ Wk ; d[b] = scale[b]*bk + shift[b], float64."""
    c = c.astype(np.float64)
    scale = 1.0 / (1.0 + np.exp(-(c @ Wsk.astype(np.float64).T
                                  + bsk.astype(np.float64))))     # [B,3]
    shift = c @ Wbk.astype(np.float64).T + bbk.astype(np.float64)  # [B,3]
    A = scale[:, :, None] * Wk.astype(np.float64)[None]            # [B,3,3]
    d = scale * bk.astype(np.float64) + shift                      # [B,3]
    return A, d


def kernel(t, x, c,
           W0, b0, Ws0, bs0, Wb0, bb0,
           W1, b1, Ws1, bs1, Wb1, bb1,
           W2, b2, Ws2, bs2, Wb2, bb2):
    global LAST_EXEC_NS
    if PROFILE:
        _install_profile_shim()
    from concourse.bass_utils import run_bass_kernel_spmd

    x = np.asarray(x)
    c = np.asarray(c)
    (W0, b0, Ws0, bs0, Wb0, bb0, W1, b1, Ws1, bs1, Wb1, bb1,
     W2, b2, Ws2, bs2, Wb2, bb2) = (
        np.asarray(a) for a in (W0, b0, Ws0, bs0, Wb0, bb0,
                                W1, b1, Ws1, bs1, Wb1, bb1,
                                W2, b2, Ws2, bs2, Wb2, bb2))
    out_dtype = x.dtype

    key = (RAW, MM_EXACT)
    if key not in _CACHE:
        _CACHE[key] = (_build_program_raw() if RAW else _build_program())
    nc = _CACHE[key]

    # ---- host: FiLM affine params per (weight-set, batch), float64 ----
    sets = [
        _film_params(c, W0, b0, Ws0, bs0, Wb0, bb0),
        _film_params(c, W1, b1, Ws1, bs1, Wb1, bb1),
        _film_params(c, W2, b2, Ws2, bs2, Wb2, bb2),
    ]

    # ---- host: shard + relayout x ----
    # [B, N, 3] -> per core [128, L]: p = b*32 + comp*10 + g
    xp = np.ascontiguousarray(x, dtype=np.float32)
    # [B, 3, G, L]
    xt = np.ascontiguousarray(xp.transpose(0, 2, 1)).reshape(B, D, G, L)

    in_maps = []
    for cc in range(NCORES):
        bs = range(cc * BPC, (cc + 1) * BPC)
        X = np.zeros((BPC, 32, L), np.float32)
        for i, b in enumerate(bs):
            X[i, :30] = xt[b].reshape(30, L)
        W6 = np.zeros((3, P, P), np.float32)
        D128 = np.zeros((P, 3), np.float32)
        for k in range(3):
            A, dv = sets[k]
            for i, b in enumerate(bs):
                for ci_ in range(3):
                    for cj in range(3):
                        a = np.float32(A[b, ci_, cj])
                        for g in range(G):
                            W6[k, i * 32 + cj * G + g, i * 32 + ci_ * G + g] = a
                    D128[i * 32 + ci_ * G:i * 32 + ci_ * G + G, k] = \
                        np.float32(dv[b, ci_])
        in_maps.append({"x": X.reshape(P, L), "w": W6, "d": D128})

    res = run_bass_kernel_spmd(nc, in_maps, list(range(NCORES)),
                               trace=bool(PROFILE))
    if PROFILE:
        LAST_EXEC_NS = res.exec_time_ns

    # ---- host: gather + inverse layout ----
    out = np.empty((B, N, D), out_dtype)
    for cc in range(NCORES):
        Y = res.results[cc]["y"].reshape(BPC, 32, L)
        for i in range(BPC):
            b = cc * BPC + i
            # [30, L] -> [3, NPAD] -> [NPAD, 3] -> [:N]
            yb = Y[i, :30].reshape(D, NPAD)
            out[b] = yb.T[:N].astype(out_dtype, copy=False)
    return out


# revision 35
# speedup vs baseline: 1.0163x; 1.0163x over previous
"""Trainium2 Bass kernel for nn_FCond (FiLM-conditioned MLP chain).

Reference computation (B=32, N=100000, D=3, CDIM=128):
    h = x
    for kblk in [0, 1, 2, 2, 2, 2]:
        h = tanh((h @ Wk.T + bk) * sigmoid(c @ Wsk.T + bsk) + (c @ Wbk.T + bbk))

Since the FiLM conditioning depends only on (c, weights), each (batch,
block) reduces to an affine map  h' = tanh(A_kb @ h + d_kb)  with
A_kb [3,3], d_kb [3] precomputed on the host in float64.

Device strategy (pure data parallel over 8 cores, 4 batches/core):
  - Layout: partition p = b*32 + comp*10 + g  (4 batch-bands of 32
    partitions; 3 comps x 10 point-groups per band; rows 30,31 of each
    band are zero padding). Free dim = 10240 points per (b,comp,g)
    stream (N padded 100000 -> 102400).
  - Each block is ONE block-diagonal [128x128] matmul on TensorE
    (40 real points per column), PSUM accumulated, then ScalarE does
    tanh(psum + d) with a per-partition bias AP, evacuating PSUM->SBUF.
  - 5 chunks of 2048 columns stream through DMA-in -> 6 blocks -> DMA-out,
    interleaved in groups of 3 chunks so consecutive PE matmul groups come
    from independent chunks (ScalarE tanh is the bottleneck engine; the
    PE/ACT chain stays dense). A 16-matmul warmup burst flips the PE HAM
    clock gate to 2.4 GHz before the main chain.

MM_DTYPE: float32r (TF32-like reduced-precision PE mode, ~1.9 cyc/col
warm) vs float32 (exact, ~4.5 cyc/col). Selected by MM_EXACT below.
Measured end-to-end rel err vs the fp32 reference: 8.8e-5 (f32r).
"""
import sys
import types

import numpy as np

B, N, D, CDIM = 32, 100000, 3, 128
NCORES = 8
BPC = B // NCORES          # batches per core
G = 10                     # point-groups per (batch, comp)
L = 10000                  # points per partition stream (N / G, exact)
NPAD = G * L               # = N exactly (no padding)
P = 128                    # partitions
CHUNK = 2048               # free-dim chunk (4 PSUM banks fp32)
MM_F = 512                 # matmul free chunk (1 PSUM bank)
NCHUNK = 5                 # chunk sizes: 4x2048 + 1808

MM_EXACT = False           # True -> float32 matmuls (exact, ~2.9x slower PE)
RAW = False                # hand-scheduled pipeline (no TileContext); Tile version is faster

PROFILE = False            # set by test harness; collects HW exec time
LAST_EXEC_NS = None

_CACHE = {}


def _install_profile_shim():
    """Register the NTFF profile hook (missing antenv.axon_hooks in this
    container) so run_bass_kernel_spmd(trace=True) can report exec time."""
    if "antenv.axon_hooks" in sys.modules:
        return
    mod = types.ModuleType("antenv.axon_hooks")
    _state = {"hook": None}
    mod.set_axon_ntff_profile_hook = lambda h: _state.__setitem__("hook", h)
    mod.get_axon_ntff_profile_hook = lambda: _state["hook"]
    sys.modules["antenv.axon_hooks"] = mod
    try:
        from trn_agent_boot.trn_boot import _ntff_profile_via_ctypes
        mod.set_axon_ntff_profile_hook(
            _ntff_profile_via_ctypes("/opt/axon/libaxon_pjrt.so"))
    except Exception:
        pass
    import concourse.bass_utils as bu
    bu.upload_artifacts = lambda tmpdir: f"local:{tmpdir}"


def _build_program():
    import concourse.bacc as bacc
    import concourse.tile as tile
    from concourse import mybir

    f32 = mybir.dt.float32
    mmdt = f32 if MM_EXACT else mybir.dt.float32r
    Tanh = mybir.ActivationFunctionType.Tanh
    Copy = mybir.ActivationFunctionType.Copy
    WSETS = (0, 1, 2, 2, 2, 2)

    nc = bacc.Bacc("TRN2", target_bir_lowering=False, debug=False)
    x_d = nc.declare_dram_parameter("x", [P, L], f32, isOutput=False)
    w_d = nc.declare_dram_parameter("w", [3, P, P], f32, isOutput=False)
    d_d = nc.declare_dram_parameter("d", [P, 3], f32, isOutput=False)
    y_d = nc.declare_dram_parameter("y", [P, L], f32, isOutput=True)

    with tile.TileContext(nc) as tc:
        with (
            tc.tile_pool(name="wpool", bufs=1) as wpool,
            tc.tile_pool(name="xinpool", bufs=5) as xinpool,
            tc.tile_pool(name="youtpool", bufs=5) as youtpool,
            tc.tile_pool(name="hpool", bufs=6) as hpool,
            tc.tile_pool(name="psum", bufs=2, space="PSUM") as psum,
        ):
            # --- first compute chunk's DMA goes out before anything else
            # so the PE/ACT chain can start ASAP. ---
            h0 = xinpool.tile([P, CHUNK], mmdt, name="xin0", tag="xin")
            nc.sync.dma_start(h0[:], x_d[:, 0:CHUNK].bitcast(mmdt))

            # --- weights/bias: DMA once, one ACT-copy (f32r rounding +
            # makes matmul weight input ACT-produced). ---
            bias = wpool.tile([P, 3], f32)
            nc.sync.dma_start(bias[:], d_d[:])
            wraw = wpool.tile([P, 3 * P], f32, name="wraw", tag="wraw")
            for k in range(3):
                nc.sync.dma_start(wraw[:, k * P:(k + 1) * P], w_d[k])
            wall = wpool.tile([P, 3 * P], mmdt, name="wall", tag="wall")
            nc.scalar.activation(wall[:], wraw[:], Copy)
            wts = [wall[:, k * P:(k + 1) * P] for k in range(3)]

            # PE warmup burst: ~16 dense matmuls (~4us) to flip the HAM
            # clock gate to 2.4 GHz before the main chain; runs while the
            # input DMAs stream in.
            warm0 = wpool.tile([P, MM_F], f32, name="warm0", tag="warm0")
            nc.vector.memset(warm0[:], 0.0)
            # dummy tanh: pulls the ACT table load off the critical chain
            nc.scalar.activation(warm0[:, 0:1], warm0[:, 0:1], Tanh,
                                 bias=0.0, scale=1.0)
            warm_src = wpool.tile([P, MM_F], mmdt, name="warmsrc",
                                  tag="warmsrc")
            nc.scalar.activation(warm_src[:], warm0[:], Copy)
            warm_ps = psum.tile([P, MM_F], f32, name="warmps", tag="ps")
            for _ in range(16):
                nc.tensor.matmul(warm_ps[:], warm_src[:, 0:P], warm_src[:],
                                 start=True, stop=True)

            # Chunk-group software pipeline: within a group, consecutive
            # matmul groups come from rotating chunks, so each group's
            # dependency on the previous block's tanh has >=2 matmul
            # groups of slack and the PE streams. First chunk is small so
            # the chain starts as soon as its DMA lands.
            sizes = [CHUNK] * (NCHUNK - 1) + [L - CHUNK * (NCHUNK - 1)]
            offs = [sum(sizes[:i]) for i in range(len(sizes))]
            groups = [(0, 1, 2), (3, 4)]
            hs = {}
            hs[0] = h0
            for grp in groups:
                for ci in grp:
                    if ci == 0:
                        continue
                    h = xinpool.tile([P, sizes[ci]], mmdt, name=f"xin{ci}",
                                     tag="xin")
                    nc.sync.dma_start(
                        h[:],
                        x_d[:, offs[ci]:offs[ci] + sizes[ci]].bitcast(mmdt))
                    hs[ci] = h
                for kblk in range(6):
                    ks = WSETS[kblk]
                    last = kblk == 5
                    for ci in grp:
                        sz = sizes[ci]
                        ps = psum.tile([P, sz], f32,
                                       name=f"ps{ci}_{kblk}", tag="ps")
                        for j in range(0, sz, MM_F):
                            je = min(j + MM_F, sz)
                            nc.tensor.matmul(
                                ps[:, j:je],
                                wts[ks],
                                hs[ci][:, j:je],
                                start=True, stop=True)
                        hn = (youtpool.tile([P, sz], f32,
                                            name=f"yo{ci}", tag="yout")
                              if last else
                              hpool.tile([P, sz], mmdt,
                                         name=f"h{ci}_{kblk}", tag="h"))
                        nc.scalar.activation(hn[:], ps[:], Tanh,
                                             bias=bias[:, ks:ks + 1],
                                             scale=1.0)
                        hs[ci] = hn
                        if last:
                            c0 = offs[ci]
                            nc.sync.dma_start(y_d[:, c0:c0 + sz], hn[:])
    nc.compile()
    return nc


def _build_program_raw():
    """Hand-scheduled variant (no TileContext): manual semaphores, static
    SBUF layout. Avoids Tile's ~2.7us preamble and ~13us epilogue
    (drain + all-engine barriers + 57-semaphore clear butterfly).

    Pipeline: 30 stages s = (group, kblk, chunk), chunk-interleaved as
    [(0,1,2),(3,4)]. PSUM ping-pongs 2 x [128,2048] (banks 0-3 / 4-7).
      PE  stage s: wait tanh(input stage) and tanh(s-2) (psum WAR), 4 MMs,
                   inc pe_sem.
      ACT stage s: wait pe_sem >= s+1, tanh(ps[s%2] + bias) -> h, inc.
      SYNC: all input DMAs up front; per-chunk output DMA after its last
            tanh; final wait for DMA completion.
    """
    import concourse.bass as bass
    import concourse.bacc as bacc
    from concourse import mybir

    f32 = mybir.dt.float32
    mmdt = f32 if MM_EXACT else mybir.dt.float32r
    Tanh = mybir.ActivationFunctionType.Tanh
    Copy = mybir.ActivationFunctionType.Copy
    WSETS = (0, 1, 2, 2, 2, 2)

    nc = bacc.Bacc("TRN2", target_bir_lowering=False, debug=False)
    x_d = nc.declare_dram_parameter("x", [P, L], f32, isOutput=False)
    w_d = nc.declare_dram_parameter("w", [3, P, P], f32, isOutput=False)
    d_d = nc.declare_dram_parameter("d", [P, 3], f32, isOutput=False)
    y_d = nc.declare_dram_parameter("y", [P, L], f32, isOutput=True)

    # static SBUF tensors
    xin = [nc.alloc_sbuf_tensor(f"xin{c}", [P, CHUNK], mmdt).ap()
           for c in range(NCHUNK)]
    ha = [nc.alloc_sbuf_tensor(f"ha{c}", [P, CHUNK], mmdt).ap()
          for c in range(NCHUNK)]
    hb = [nc.alloc_sbuf_tensor(f"hb{c}", [P, CHUNK], mmdt).ap()
          for c in range(NCHUNK)]
    yout = [nc.alloc_sbuf_tensor(f"yout{c}", [P, CHUNK], f32).ap()
            for c in range(NCHUNK)]
    wraw = nc.alloc_sbuf_tensor("wraw", [P, 3 * P], f32).ap()
    wall = nc.alloc_sbuf_tensor("wall", [P, 3 * P], mmdt).ap()
    biast = nc.alloc_sbuf_tensor("biast", [P, 3], f32).ap()
    ps = [nc.alloc_psum_tensor(f"ps{i}", [P, CHUNK], f32).ap()
          for i in range(2)]

    # stage enumeration
    stages = []
    for grp in ((0, 1, 2), (3, 4)):
        for kblk in range(6):
            for ci in grp:
                stages.append((kblk, ci))
    idx = {kc: s for s, kc in enumerate(stages)}

    # h buffer chain per chunk: block k's input; k=0 -> xin, then a/b
    def h_in(kblk, ci):
        if kblk == 0:
            return xin[ci]
        return ha[ci] if kblk % 2 == 1 else hb[ci]

    def h_out(kblk, ci):
        if kblk == 5:
            return yout[ci]
        return ha[ci] if kblk % 2 == 0 else hb[ci]

    with (
        nc.Block() as block,
        nc.semaphore("dma_x0") as dx0,
        nc.semaphore("dma_x1") as dx1,
        nc.semaphore("dma_x2") as dx2,
        nc.semaphore("dma_x3") as dx3,
        nc.semaphore("dma_x4") as dx4,
        nc.semaphore("dma_wd") as dwd,
        nc.semaphore("dma_out") as dout,
        nc.semaphore("act_sem") as act_sem,
        nc.semaphore("pe_sem") as pe_sem,
    ):
        dxs = [dx0, dx1, dx2, dx3, dx4]

        @block.sync
        def _(sync: bass.BassEngine):
            sync.dma_start(out=biast, in_=d_d[:]).then_inc(dwd, 16)
            for k in range(3):
                sync.dma_start(out=wraw[:, k * P:(k + 1) * P],
                               in_=w_d[k]).then_inc(dwd, 16)
            sync.dma_start(out=xin[0], in_=x_d[:, 0:CHUNK].bitcast(mmdt)
                           ).then_inc(dxs[0], 16)
            for c in range(1, NCHUNK):
                if c >= 2:
                    # bound in-flight HWDGE transfers (ring capacity)
                    sync.wait_ge(dxs[c - 2], 16)
                sync.dma_start(out=xin[c],
                               in_=x_d[:, c * CHUNK:(c + 1) * CHUNK]
                               .bitcast(mmdt)).then_inc(dxs[c], 16)
            for ci in range(NCHUNK):
                sync.dma_start(out=y_d[:, ci * CHUNK:(ci + 1) * CHUNK],
                               in_=h_out(5, ci)
                               )._wait_ge(act_sem, idx[(5, ci)] + 2
                                          ).then_inc(dout, 16)
            sync.wait_ge(dout, 16 * NCHUNK)

        @block.scalar
        def _(scalar: bass.BassEngine):
            scalar.wait_ge(dwd, 64)
            scalar.activation(wall, wraw, Copy).then_inc(act_sem, 1)
            for s, (kblk, ci) in enumerate(stages):
                ks = WSETS[kblk]
                out_ap = h_out(kblk, ci)
                scalar.activation(
                    out_ap, ps[s % 2], Tanh,
                    bias=biast[:, ks:ks + 1], scale=1.0,
                )._wait_ge(pe_sem, s + 1).then_inc(act_sem, 1)

        @block.tensor
        def _(tensor: bass.BassEngine):
            tensor.wait_ge(act_sem, 1)
            for _i in range(16):
                tensor.matmul(ps[0][:, 0:P], wall[:, 0:P], wall[:, 0:P],
                              start=True, stop=True)
            for s, (kblk, ci) in enumerate(stages):
                ks = WSETS[kblk]
                # input-ready + psum-WAR deps, folded to one act_sem wait
                a_req = 0
                if kblk > 0:
                    a_req = idx[(kblk - 1, ci)] + 2
                if s >= 2:
                    a_req = max(a_req, s)
                if kblk == 0:
                    tensor.wait_ge(dxs[ci], 16)
                rhs = h_in(kblk, ci)
                for j in range(0, CHUNK, MM_F):
                    mm = tensor.matmul(ps[s % 2][:, j:j + MM_F],
                                       wall[:, ks * P:(ks + 1) * P],
                                       rhs[:, j:j + MM_F],
                                       start=True, stop=True)
                    if j == 0 and a_req:
                        mm._wait_ge(act_sem, a_req)
                mm.then_inc(pe_sem, 1)

    nc.compile()
    return nc


def _film_params(c, Wk, bk, Wsk, bsk, Wbk, bbk):
    """A[b] = diag(scale[b]) @ Wk ; d[b] = scale[b]*bk + shift[b], float64."""
    c = c.astype(np.float64)
    scale = 1.0 / (1.0 + np.exp(-(c @ Wsk.astype(np.float64).T
                                  + bsk.astype(np.float64))))     # [B,3]
    shift = c @ Wbk.astype(np.float64).T + bbk.astype(np.float64)  # [B,3]
    A = scale[:, :, None] * Wk.astype(np.float64)[None]            # [B,3,3]
    d = scale * bk.astype(np.float64) + shift                      # [B,3]
    return A, d


def kernel(t, x, c,
           W0, b0, Ws0, bs0, Wb0, bb0,
           W1, b1, Ws1, bs1, Wb1, bb1,
           W2, b2, Ws2, bs2, Wb2, bb2):
    global LAST_EXEC_NS
    if PROFILE:
        _install_profile_shim()
    from concourse.bass_utils import run_bass_kernel_spmd

    x = np.asarray(x)
    c = np.asarray(c)
    (W0, b0, Ws0, bs0, Wb0, bb0, W1, b1, Ws1, bs1, Wb1, bb1,
     W2, b2, Ws2, bs2, Wb2, bb2) = (
        np.asarray(a) for a in (W0, b0, Ws0, bs0, Wb0, bb0,
                                W1, b1, Ws1, bs1, Wb1, bb1,
                                W2, b2, Ws2, bs2, Wb2, bb2))
    out_dtype = x.dtype

    key = (RAW, MM_EXACT)
    if key not in _CACHE:
        _CACHE[key] = (_build_program_raw() if RAW else _build_program())
    nc = _CACHE[key]

    # ---- host: FiLM affine params per (weight-set, batch), float64 ----
    sets = [
        _film_params(c, W0, b0, Ws0, bs0, Wb0, bb0),
        _film_params(c, W1, b1, Ws1, bs1, Wb1, bb1),
        _film_params(c, W2, b2, Ws2, bs2, Wb2, bb2),
    ]

    # ---- host: shard + relayout x ----
    # [B, N, 3] -> per core [128, L]: p = b*32 + comp*10 + g
    xp = np.ascontiguousarray(x, dtype=np.float32)
    # [B, 3, G, L]
    xt = np.ascontiguousarray(xp.transpose(0, 2, 1)).reshape(B, D, G, L)

    in_maps = []
    for cc in range(NCORES):
        bs = range(cc * BPC, (cc + 1) * BPC)
        X = np.zeros((BPC, 32, L), np.float32)
        for i, b in enumerate(bs):
            X[i, :30] = xt[b].reshape(30, L)
        W6 = np.zeros((3, P, P), np.float32)
        D128 = np.zeros((P, 3), np.float32)
        for k in range(3):
            A, dv = sets[k]
            for i, b in enumerate(bs):
                for ci_ in range(3):
                    for cj in range(3):
                        a = np.float32(A[b, ci_, cj])
                        for g in range(G):
                            W6[k, i * 32 + cj * G + g, i * 32 + ci_ * G + g] = a
                    D128[i * 32 + ci_ * G:i * 32 + ci_ * G + G, k] = \
                        np.float32(dv[b, ci_])
        in_maps.append({"x": X.reshape(P, L), "w": W6, "d": D128})

    res = run_bass_kernel_spmd(nc, in_maps, list(range(NCORES)),
                               trace=bool(PROFILE))
    if PROFILE:
        LAST_EXEC_NS = res.exec_time_ns

    # ---- host: gather + inverse layout ----
    out = np.empty((B, N, D), out_dtype)
    for cc in range(NCORES):
        Y = res.results[cc]["y"].reshape(BPC, 32, L)
        for i in range(BPC):
            b = cc * BPC + i
            # [30, L] -> [3, NPAD] -> [NPAD, 3] -> [:N]
            yb = Y[i, :30].reshape(D, NPAD)
            out[b] = yb.T[:N].astype(out_dtype, copy=False)
    return out
